# revision 3
# baseline (speedup 1.0000x reference)
"""Corotational 2D beam kernel for 8 trn2 NeuronCores.

Strategy (element sharding across 8 cores):
- Host prepares per-element input planes (node displacements/coords gathered
  per endpoint via the connectivity index array + material props) padded to a
  power-of-two tile grid, packed as one DRAM blob per core.
- Each core streams its element tiles, computes the full corotational beam
  force pipeline (rotation to local frame, local stiffness forces, rotation
  back) on the vector/scalar engines, and streams all 15 per-element outputs
  back as one contiguous blob per tile.
- nodal_forces (scatter-add of the computed f_global over the 2M nodes) is
  reduced on the host from the device-computed f_global columns.
"""
import numpy as np

import concourse.bacc as bacc
import concourse.bass as bass
import concourse.tile as tile
import concourse.mybir as mybir
from concourse.bass_utils import run_bass_kernel_spmd

N_NODES = 2_000_000
N_ELEMS = 4_000_000
NCORES = 8
K = 128            # elements per partition per tile
NT = 32            # tiles per core
EPC = 128 * K * NT      # elements per core (padded): 524288
EPAD = EPC * NCORES     # 4194304

IN_PLANES = 13     # uxA uyA thA xA zA uxB uyB thB xB zB pE pA pI
OUT_W = 35 * K     # 4 six-wide outputs (6K each) + 11 scalar planes

_nc_cache = None


def _build_program():
    F32 = mybir.dt.float32
    AF = mybir.ActivationFunctionType
    OP = mybir.AluOpType

    nc = bacc.Bacc("TRN2", target_bir_lowering=False, debug=False, num_devices=NCORES)
    in_blob = nc.dram_tensor("in_blob", [NT, 128, IN_PLANES * K], F32, kind="ExternalInput").ap()
    out_blob = nc.dram_tensor("out_blob", [NT, 128, OUT_W], F32, kind="ExternalOutput").ap()

    with tile.TileContext(nc) as tc:
        with tc.tile_pool(name="io", bufs=2) as iop, \
             tc.tile_pool(name="pl", bufs=2) as plp, \
             tc.tile_pool(name="tmp", bufs=4) as tmpp:

            def plane(tag):
                return plp.tile([128, K], F32, tag=tag, name=tag)

            def tmp():
                return tmpp.tile([128, K], F32, tag="tmp", name="tmp")

            for t in range(NT):
                tin = iop.tile([128, IN_PLANES * K], F32, tag="tin", name="tin")
                nc.sync.dma_start(out=tin[:], in_=in_blob[t])
                tout = iop.tile([128, OUT_W], F32, tag="tout", name="tout")

                def ip(i):  # input plane view
                    return tin[:, i * K:(i + 1) * K]

                uxA, uyA, thA, xA, zA = ip(0), ip(1), ip(2), ip(3), ip(4)
                uxB, uyB, thB, xB, zB = ip(5), ip(6), ip(7), ip(8), ip(9)
                pE, pA, pI = ip(10), ip(11), ip(12)

                V = nc.vector
                S = nc.scalar

                dx = plane("dx"); V.tensor_tensor(out=dx[:], in0=xB, in1=xA, op=OP.subtract)
                dz = plane("dz"); V.tensor_tensor(out=dz[:], in0=zB, in1=zA, op=OP.subtract)
                l2 = plane("l2"); V.tensor_tensor(out=l2[:], in0=dx[:], in1=dx[:], op=OP.mult)
                t0 = tmp(); V.tensor_tensor(out=t0[:], in0=dz[:], in1=dz[:], op=OP.mult)
                V.tensor_tensor(out=l2[:], in0=l2[:], in1=t0[:], op=OP.add)
                l0v = plane("l0v"); S.activation(out=l0v[:], in_=l2[:], func=AF.Sqrt)
                inv = plane("inv"); V.reciprocal(out=inv[:], in_=l0v[:])
                cc = plane("cc"); V.tensor_tensor(out=cc[:], in0=dx[:], in1=inv[:], op=OP.mult)
                ss = plane("ss"); V.tensor_tensor(out=ss[:], in0=dz[:], in1=inv[:], op=OP.mult)

                EAL = plane("EAL")
                V.tensor_tensor(out=EAL[:], in0=pE, in1=pA, op=OP.mult)
                V.tensor_tensor(out=EAL[:], in0=EAL[:], in1=inv[:], op=OP.mult)
                EIL = plane("EIL")
                V.tensor_tensor(out=EIL[:], in0=pE, in1=pI, op=OP.mult)
                V.tensor_tensor(out=EIL[:], in0=EIL[:], in1=inv[:], op=OP.mult)
                EIL2 = plane("EIL2"); V.tensor_tensor(out=EIL2[:], in0=EIL[:], in1=inv[:], op=OP.mult)
                EIL3 = plane("EIL3"); V.tensor_tensor(out=EIL3[:], in0=EIL2[:], in1=inv[:], op=OP.mult)

                # rotate displacements to local frame
                def rot(out_ap, a_ap, b_ap, opb):
                    m1 = tmp(); V.tensor_tensor(out=m1[:], in0=cc[:], in1=a_ap, op=OP.mult)
                    m2 = tmp(); V.tensor_tensor(out=m2[:], in0=ss[:], in1=b_ap, op=OP.mult)
                    V.tensor_tensor(out=out_ap, in0=m1[:], in1=m2[:], op=opb)

                ua = plane("ua"); rot(ua[:], uxA, uyA, OP.add)           # c*ux + s*uy
                ub = plane("ub"); rot(ub[:], uxB, uyB, OP.add)
                wa = plane("wa")
                m1 = tmp(); V.tensor_tensor(out=m1[:], in0=cc[:], in1=uyA, op=OP.mult)
                m2 = tmp(); V.tensor_tensor(out=m2[:], in0=ss[:], in1=uxA, op=OP.mult)
                V.tensor_tensor(out=wa[:], in0=m1[:], in1=m2[:], op=OP.subtract)  # c*uy - s*ux
                wb = plane("wb")
                m1 = tmp(); V.tensor_tensor(out=m1[:], in0=cc[:], in1=uyB, op=OP.mult)
                m2 = tmp(); V.tensor_tensor(out=m2[:], in0=ss[:], in1=uxB, op=OP.mult)
                V.tensor_tensor(out=wb[:], in0=m1[:], in1=m2[:], op=OP.subtract)

                ud = plane("ud"); V.tensor_tensor(out=ud[:], in0=ua[:], in1=ub[:], op=OP.subtract)
                wd = plane("wd"); V.tensor_tensor(out=wd[:], in0=wa[:], in1=wb[:], op=OP.subtract)
                ts = plane("ts"); V.tensor_tensor(out=ts[:], in0=thA, in1=thB, op=OP.add)

                f0 = plane("f0"); V.tensor_tensor(out=f0[:], in0=EAL[:], in1=ud[:], op=OP.mult)
                f3 = plane("f3"); V.tensor_scalar_mul(f3[:], f0[:], -1.0)

                av = plane("av"); V.tensor_tensor(out=av[:], in0=EIL3[:], in1=wd[:], op=OP.mult)
                bv = plane("bv"); V.tensor_tensor(out=bv[:], in0=EIL2[:], in1=ts[:], op=OP.mult)
                f1 = plane("f1")
                t1 = tmp(); V.tensor_scalar_mul(t1[:], av[:], 12.0)
                t2 = tmp(); V.tensor_scalar_mul(t2[:], bv[:], 6.0)
                V.tensor_tensor(out=f1[:], in0=t1[:], in1=t2[:], op=OP.add)
                f4 = plane("f4"); V.tensor_scalar_mul(f4[:], f1[:], -1.0)

                c16 = plane("c16")
                V.tensor_tensor(out=c16[:], in0=EIL2[:], in1=wd[:], op=OP.mult)
                V.tensor_scalar_mul(c16[:], c16[:], 6.0)
                # f2 = c16 + EIL*(4 ta + 2 tb);  f5 = c16 + EIL*(2 ta + 4 tb)
                f2 = plane("f2")
                r1 = tmp(); V.tensor_scalar_mul(r1[:], thA, 2.0)
                r2 = tmp(); V.tensor_tensor(out=r2[:], in0=r1[:], in1=thB, op=OP.add)
                V.tensor_tensor(out=r2[:], in0=r2[:], in1=EIL[:], op=OP.mult)
                V.tensor_scalar_mul(r2[:], r2[:], 2.0)
                V.tensor_tensor(out=f2[:], in0=c16[:], in1=r2[:], op=OP.add)
                f5 = plane("f5")
                r3 = tmp(); V.tensor_scalar_mul(r3[:], thB, 2.0)
                r4 = tmp(); V.tensor_tensor(out=r4[:], in0=r3[:], in1=thA, op=OP.add)
                V.tensor_tensor(out=r4[:], in0=r4[:], in1=EIL[:], op=OP.mult)
                V.tensor_scalar_mul(r4[:], r4[:], 2.0)
                V.tensor_tensor(out=f5[:], in0=c16[:], in1=r4[:], op=OP.add)

                g0 = plane("g0")
                m1 = tmp(); V.tensor_tensor(out=m1[:], in0=cc[:], in1=f0[:], op=OP.mult)
                m2 = tmp(); V.tensor_tensor(out=m2[:], in0=ss[:], in1=f1[:], op=OP.mult)
                V.tensor_tensor(out=g0[:], in0=m1[:], in1=m2[:], op=OP.subtract)
                g1 = plane("g1")
                m1 = tmp(); V.tensor_tensor(out=m1[:], in0=ss[:], in1=f0[:], op=OP.mult)
                m2 = tmp(); V.tensor_tensor(out=m2[:], in0=cc[:], in1=f1[:], op=OP.mult)
                V.tensor_tensor(out=g1[:], in0=m1[:], in1=m2[:], op=OP.add)
                g3 = plane("g3"); V.tensor_scalar_mul(g3[:], g0[:], -1.0)
                g4 = plane("g4"); V.tensor_scalar_mul(g4[:], g1[:], -1.0)

                ul = plane("ul"); V.tensor_scalar_mul(ul[:], ud[:], -1.0)
                phi = plane("phi")
                V.tensor_tensor(out=phi[:], in0=wd[:], in1=inv[:], op=OP.mult)
                V.tensor_scalar_mul(phi[:], phi[:], -1.0)

                # ---- stage outputs into tout ----
                def six(base, comps):
                    reg = tout[:, base * K:(base + 6) * K].rearrange("p (j c) -> p j c", c=6)
                    for ci, ap in enumerate(comps):
                        eng = S if ci % 2 == 0 else V
                        if eng is S:
                            S.activation(out=reg[:, :, ci], in_=ap, func=AF.Copy)
                        else:
                            V.tensor_copy(out=reg[:, :, ci], in_=ap)

                six(0, [g0[:], g1[:], f2[:], g3[:], g4[:], f5[:]])             # f_global
                six(6, [f0[:], f1[:], f2[:], f3[:], f4[:], f5[:]])             # f_local
                six(12, [ua[:], wa[:], thA, ub[:], wb[:], thB])                # d_local
                six(18, [uxA, uyA, thA, uxB, uyB, thB])                        # d_global

                scal = [f3[:], f4[:], f2[:], f5[:], ul[:], thA, thB, phi[:], l0v[:], cc[:], ss[:]]
                for i, ap in enumerate(scal):
                    V.tensor_copy(out=tout[:, (24 + i) * K:(25 + i) * K], in_=ap)

                nc.sync.dma_start(out=out_blob[t], in_=tout[:])

    nc.compile()
    return nc


def kernel(pred_disp, coords, conn, prop_E, prop_A, prop_I22):
    global _nc_cache
    if _nc_cache is None:
        _nc_cache = _build_program()
    nc = _nc_cache

    pred_disp = np.asarray(pred_disp, dtype=np.float32)
    coords = np.asarray(coords, dtype=np.float32)
    conn = np.asarray(conn)
    prop_E = np.asarray(prop_E, dtype=np.float32)
    prop_A = np.asarray(prop_A, dtype=np.float32)
    prop_I22 = np.asarray(prop_I22, dtype=np.float32)
    E = conn.shape[0]
    nA = conn[:, 0].astype(np.int64)
    nB = conn[:, 1].astype(np.int64)

    # host-side shard prep: gather per-endpoint node planes + pad to tile grid
    planes = np.zeros((IN_PLANES, EPAD), dtype=np.float32)
    planes[0, :E] = pred_disp[nA, 0]
    planes[1, :E] = pred_disp[nA, 1]
    planes[2, :E] = pred_disp[nA, 2]
    planes[3, :E] = coords[nA, 0]
    planes[4, :E] = coords[nA, 2]
    planes[5, :E] = pred_disp[nB, 0]
    planes[6, :E] = pred_disp[nB, 1]
    planes[7, :E] = pred_disp[nB, 2]
    planes[8, :E] = coords[nB, 0]
    planes[9, :E] = coords[nB, 2]
    planes[8, E:] = 1.0  # pad elements: xB - xA = 1 -> l0 = 1, everything else 0
    planes[10, :E] = prop_E
    planes[11, :E] = prop_A
    planes[12, :E] = prop_I22

    # blob layout: e = ((c*NT + t)*128 + P)*K + j
    in_all = np.ascontiguousarray(
        planes.reshape(IN_PLANES, NCORES, NT, 128, K).transpose(1, 2, 3, 0, 4)
    ).reshape(NCORES, NT, 128, IN_PLANES * K)

    in_maps = [{"in_blob": in_all[c]} for c in range(NCORES)]
    res = run_bass_kernel_spmd(nc, in_maps, list(range(NCORES)))
    big = np.stack([res.results[c]["out_blob"] for c in range(NCORES)])  # [8, NT, 128, OUT_W]

    def six_out(base):
        reg = big[:, :, :, base * K:(base + 6) * K]
        return np.ascontiguousarray(reg).reshape(EPAD, 6)[:E]

    f_global = six_out(0)
    f_local = six_out(6)
    d_local = six_out(12)
    d_global = six_out(18)

    def plane_out(i):
        return np.ascontiguousarray(
            big[:, :, :, (24 + i) * K:(25 + i) * K]).reshape(EPAD)[:E]

    N_e, V_e, M1_e, M2_e, u_l, ta, tb, phi, l0, c, s = (plane_out(i) for i in range(11))

    # host reduction: nodal_forces[n] = sum of f_global endpoint contributions
    nodal = np.zeros((N_NODES, 3), dtype=np.float32)
    for col in range(3):
        acc = np.bincount(nA, weights=f_global[:, col].astype(np.float64), minlength=N_NODES)
        acc += np.bincount(nB, weights=f_global[:, col + 3].astype(np.float64), minlength=N_NODES)
        nodal[:, col] = acc.astype(np.float32)

    return (nodal, f_global, f_local, d_local, d_global,
            N_e, V_e, M1_e, M2_e, u_l, ta, tb, phi, l0, c, s)


# revision 5
# speedup vs baseline: 1.2176x; 1.2176x over previous
"""Corotational 2D beam kernel for 8 trn2 NeuronCores.

Strategy (element sharding across 8 cores):
- Host prepares per-element input planes (node displacements/coords gathered
  per endpoint via the connectivity index array + material props) padded to a
  power-of-two tile grid, packed as one DRAM blob per core.
- Each core streams its element tiles and computes the corotational beam
  force pipeline (rotation to local frame, local stiffness forces, rotation
  back to global) split across the vector, scalar(ACT) and gpsimd engines,
  then streams the computed outputs back as one contiguous blob per tile.
- Outputs that are pure copies of gathered inputs (d_global, ta, tb) or
  duplicate columns of computed outputs (N_e/V_e/M1_e/M2_e = f_local cols)
  are assembled host-side; nodal_forces is reduced host-side from the
  device-computed f_global columns.
"""
import numpy as np

import concourse.bacc as bacc
import concourse.bass as bass
import concourse.tile as tile
import concourse.mybir as mybir
from concourse.bass_utils import run_bass_kernel_spmd

N_NODES = 2_000_000
N_ELEMS = 4_000_000
NCORES = 8
K = 256            # elements per partition per tile
NT = 16            # tiles per core
EPC = 128 * K * NT      # elements per core (padded): 524288
EPAD = EPC * NCORES     # 4194304

IN_PLANES = 13     # uxA uyA thA xA zA uxB uyB thB xB zB pE pA pI
# out regions (per partition, f32 words): f_global 6K | f_local 6K | d_local 6K
# | planes: ul phi l0 c s (5K)
OUT_W = 23 * K

_nc_cache = None


def _build_program():
    F32 = mybir.dt.float32
    AF = mybir.ActivationFunctionType
    OP = mybir.AluOpType

    nc = bacc.Bacc("TRN2", target_bir_lowering=False, debug=False, num_devices=NCORES)
    in_blob = nc.dram_tensor("in_blob", [NT, 128, IN_PLANES * K], F32, kind="ExternalInput").ap()
    out_blob = nc.dram_tensor("out_blob", [NT, 128, OUT_W], F32, kind="ExternalOutput").ap()

    with tile.TileContext(nc) as tc:
        with tc.tile_pool(name="io", bufs=2) as iop, \
             tc.tile_pool(name="pl", bufs=2) as plp, \
             tc.tile_pool(name="tmp", bufs=4) as tmpp:

            def plane(tag):
                return plp.tile([128, K], F32, tag=tag, name=tag)

            def tmp():
                return tmpp.tile([128, K], F32, tag="tmp", name="tmp")

            for t in range(NT):
                tin = iop.tile([128, IN_PLANES * K], F32, tag="tin", name="tin")
                nc.sync.dma_start(out=tin[:], in_=in_blob[t])
                tout = iop.tile([128, OUT_W], F32, tag="tout", name="tout")

                def ip(i):  # input plane view
                    return tin[:, i * K:(i + 1) * K]

                uxA, uyA, thA, xA, zA = ip(0), ip(1), ip(2), ip(3), ip(4)
                uxB, uyB, thB, xB, zB = ip(5), ip(6), ip(7), ip(8), ip(9)
                pE, pA, pI = ip(10), ip(11), ip(12)

                V = nc.vector
                S = nc.scalar
                G = nc.gpsimd

                # --- geometry (DVE + ACT sqrt) ---
                dx = plane("dx"); G.tensor_tensor(out=dx[:], in0=xB, in1=xA, op=OP.subtract)
                dz = plane("dz"); G.tensor_tensor(out=dz[:], in0=zB, in1=zA, op=OP.subtract)
                l2 = plane("l2"); V.tensor_tensor(out=l2[:], in0=dx[:], in1=dx[:], op=OP.mult)
                t0 = tmp(); V.tensor_tensor(out=t0[:], in0=dz[:], in1=dz[:], op=OP.mult)
                V.tensor_tensor(out=l2[:], in0=l2[:], in1=t0[:], op=OP.add)
                l0v = plane("l0v"); S.activation(out=l0v[:], in_=l2[:], func=AF.Sqrt)
                inv = plane("inv"); V.reciprocal(out=inv[:], in_=l0v[:])
                cc = plane("cc"); V.tensor_tensor(out=cc[:], in0=dx[:], in1=inv[:], op=OP.mult)
                ss = plane("ss"); V.tensor_tensor(out=ss[:], in0=dz[:], in1=inv[:], op=OP.mult)

                # --- material chain (gpsimd; independent of rotations) ---
                EAL = plane("EAL")
                G.tensor_tensor(out=EAL[:], in0=pE, in1=pA, op=OP.mult)
                G.tensor_tensor(out=EAL[:], in0=EAL[:], in1=inv[:], op=OP.mult)
                EIL = plane("EIL")
                G.tensor_tensor(out=EIL[:], in0=pE, in1=pI, op=OP.mult)
                G.tensor_tensor(out=EIL[:], in0=EIL[:], in1=inv[:], op=OP.mult)
                EIL2 = plane("EIL2"); G.tensor_tensor(out=EIL2[:], in0=EIL[:], in1=inv[:], op=OP.mult)
                ts = plane("ts"); G.tensor_tensor(out=ts[:], in0=thA, in1=thB, op=OP.add)
                h1 = plane("h1"); G.tensor_tensor(out=h1[:], in0=thA, in1=ts[:], op=OP.add)
                h3 = plane("h3"); G.tensor_tensor(out=h3[:], in0=thB, in1=ts[:], op=OP.add)

                # --- rotate displacements to local frame (DVE) ---
                ua = plane("ua")
                m1 = tmp(); V.tensor_tensor(out=m1[:], in0=cc[:], in1=uxA, op=OP.mult)
                m2 = tmp(); V.tensor_tensor(out=m2[:], in0=ss[:], in1=uyA, op=OP.mult)
                V.tensor_tensor(out=ua[:], in0=m1[:], in1=m2[:], op=OP.add)
                ub = plane("ub")
                m1 = tmp(); V.tensor_tensor(out=m1[:], in0=cc[:], in1=uxB, op=OP.mult)
                m2 = tmp(); V.tensor_tensor(out=m2[:], in0=ss[:], in1=uyB, op=OP.mult)
                V.tensor_tensor(out=ub[:], in0=m1[:], in1=m2[:], op=OP.add)
                wa = plane("wa")
                m1 = tmp(); V.tensor_tensor(out=m1[:], in0=cc[:], in1=uyA, op=OP.mult)
                m2 = tmp(); V.tensor_tensor(out=m2[:], in0=ss[:], in1=uxA, op=OP.mult)
                V.tensor_tensor(out=wa[:], in0=m1[:], in1=m2[:], op=OP.subtract)
                wb = plane("wb")
                m1 = tmp(); V.tensor_tensor(out=m1[:], in0=cc[:], in1=uyB, op=OP.mult)
                m2 = tmp(); V.tensor_tensor(out=m2[:], in0=ss[:], in1=uxB, op=OP.mult)
                V.tensor_tensor(out=wb[:], in0=m1[:], in1=m2[:], op=OP.subtract)

                ud = plane("ud"); G.tensor_tensor(out=ud[:], in0=ua[:], in1=ub[:], op=OP.subtract)
                wd = plane("wd"); G.tensor_tensor(out=wd[:], in0=wa[:], in1=wb[:], op=OP.subtract)

                # --- local forces ---
                f0 = plane("f0"); V.tensor_tensor(out=f0[:], in0=EAL[:], in1=ud[:], op=OP.mult)
                pv = plane("pv"); V.tensor_tensor(out=pv[:], in0=inv[:], in1=wd[:], op=OP.mult)
                qv = plane("qv")
                V.scalar_tensor_tensor(out=qv[:], in0=pv[:], scalar=2.0, in1=ts[:],
                                       op0=OP.mult, op1=OP.add)
                rv = plane("rv"); V.tensor_tensor(out=rv[:], in0=EIL2[:], in1=qv[:], op=OP.mult)
                f1 = plane("f1"); V.tensor_scalar_mul(f1[:], rv[:], 6.0)

                c16 = plane("c16")
                V.scalar_tensor_tensor(out=c16[:], in0=wd[:], scalar=6.0, in1=EIL2[:],
                                       op0=OP.mult, op1=OP.mult)
                f2 = plane("f2")
                h2 = tmp(); V.scalar_tensor_tensor(out=h2[:], in0=h1[:], scalar=2.0, in1=EIL[:],
                                                   op0=OP.mult, op1=OP.mult)
                V.tensor_tensor(out=f2[:], in0=c16[:], in1=h2[:], op=OP.add)
                f5 = plane("f5")
                h4 = tmp(); V.scalar_tensor_tensor(out=h4[:], in0=h3[:], scalar=2.0, in1=EIL[:],
                                                   op0=OP.mult, op1=OP.mult)
                V.tensor_tensor(out=f5[:], in0=c16[:], in1=h4[:], op=OP.add)

                # --- rotate forces back to global ---
                g0 = plane("g0")
                m1 = tmp(); V.tensor_tensor(out=m1[:], in0=cc[:], in1=f0[:], op=OP.mult)
                m2 = tmp(); V.tensor_tensor(out=m2[:], in0=ss[:], in1=f1[:], op=OP.mult)
                V.tensor_tensor(out=g0[:], in0=m1[:], in1=m2[:], op=OP.subtract)
                g1 = plane("g1")
                m1 = tmp(); V.tensor_tensor(out=m1[:], in0=ss[:], in1=f0[:], op=OP.mult)
                m2 = tmp(); V.tensor_tensor(out=m2[:], in0=cc[:], in1=f1[:], op=OP.mult)
                V.tensor_tensor(out=g1[:], in0=m1[:], in1=m2[:], op=OP.add)

                # --- stage outputs into tout ---
                def vcopy(dst, src):
                    V.tensor_copy(out=dst, in_=src)

                def acopy(dst, src, scale=1.0):
                    S.activation(out=dst, in_=src, func=AF.Copy, scale=scale)

                fg = tout[:, 0:6 * K].rearrange("p (j c) -> p j c", c=6)
                vcopy(fg[:, :, 0], g0[:])
                vcopy(fg[:, :, 1], g1[:])
                vcopy(fg[:, :, 2], f2[:])
                acopy(fg[:, :, 3], g0[:], -1.0)
                acopy(fg[:, :, 4], g1[:], -1.0)
                vcopy(fg[:, :, 5], f5[:])

                fl = tout[:, 6 * K:12 * K].rearrange("p (j c) -> p j c", c=6)
                vcopy(fl[:, :, 0], f0[:])
                vcopy(fl[:, :, 1], f1[:])
                vcopy(fl[:, :, 2], f2[:])
                acopy(fl[:, :, 3], f0[:], -1.0)
                acopy(fl[:, :, 4], rv[:], -6.0)
                vcopy(fl[:, :, 5], f5[:])

                dl = tout[:, 12 * K:18 * K].rearrange("p (j c) -> p j c", c=6)
                acopy(dl[:, :, 0], ua[:])
                acopy(dl[:, :, 1], wa[:])
                acopy(dl[:, :, 2], thA)
                acopy(dl[:, :, 3], ub[:])
                acopy(dl[:, :, 4], wb[:])
                acopy(dl[:, :, 5], thB)

                def pl_out(i):
                    return tout[:, (18 + i) * K:(19 + i) * K]

                acopy(pl_out(0), ud[:], -1.0)   # u_l = ub - ua = -ud
                acopy(pl_out(1), pv[:], -1.0)   # phi = (wb-wa)/l0 = -pv
                acopy(pl_out(2), l0v[:])
                vcopy(pl_out(3), cc[:])
                vcopy(pl_out(4), ss[:])

                nc.sync.dma_start(out=out_blob[t], in_=tout[:])

    nc.compile()
    return nc


def kernel(pred_disp, coords, conn, prop_E, prop_A, prop_I22):
    global _nc_cache
    if _nc_cache is None:
        _nc_cache = _build_program()
    nc = _nc_cache

    pred_disp = np.asarray(pred_disp, dtype=np.float32)
    coords = np.asarray(coords, dtype=np.float32)
    conn = np.asarray(conn)
    prop_E = np.asarray(prop_E, dtype=np.float32)
    prop_A = np.asarray(prop_A, dtype=np.float32)
    prop_I22 = np.asarray(prop_I22, dtype=np.float32)
    E = conn.shape[0]
    nA = conn[:, 0].astype(np.int64)
    nB = conn[:, 1].astype(np.int64)

    # host-side shard prep: gather per-endpoint node planes + pad to tile grid
    planes = np.zeros((IN_PLANES, EPAD), dtype=np.float32)
    dispA = pred_disp[nA]
    dispB = pred_disp[nB]
    planes[0, :E] = dispA[:, 0]
    planes[1, :E] = dispA[:, 1]
    planes[2, :E] = dispA[:, 2]
    planes[3, :E] = coords[nA, 0]
    planes[4, :E] = coords[nA, 2]
    planes[5, :E] = dispB[:, 0]
    planes[6, :E] = dispB[:, 1]
    planes[7, :E] = dispB[:, 2]
    planes[8, :E] = coords[nB, 0]
    planes[9, :E] = coords[nB, 2]
    planes[8, E:] = 1.0  # pad elements: xB - xA = 1 -> l0 = 1, everything else 0
    planes[10, :E] = prop_E
    planes[11, :E] = prop_A
    planes[12, :E] = prop_I22

    # blob layout: e = ((c*NT + t)*128 + P)*K + j
    in_all = np.ascontiguousarray(
        planes.reshape(IN_PLANES, NCORES, NT, 128, K).transpose(1, 2, 3, 0, 4)
    ).reshape(NCORES, NT, 128, IN_PLANES * K)

    in_maps = [{"in_blob": in_all[c]} for c in range(NCORES)]
    res = run_bass_kernel_spmd(nc, in_maps, list(range(NCORES)))
    big = np.stack([res.results[c]["out_blob"] for c in range(NCORES)])  # [8, NT, 128, OUT_W]

    def six_out(base):
        reg = big[:, :, :, base * K:(base + 6) * K]
        return np.ascontiguousarray(reg).reshape(EPAD, 6)[:E]

    f_global = six_out(0)
    f_local = six_out(6)
    d_local = six_out(12)

    def plane_out(i):
        return np.ascontiguousarray(
            big[:, :, :, (18 + i) * K:(19 + i) * K]).reshape(EPAD)[:E]

    u_l, phi, l0, c, s = (plane_out(i) for i in range(5))

    # outputs that are pure copies of gathered inputs / computed columns
    d_global = np.concatenate([dispA, dispB], axis=1)
    ta = np.ascontiguousarray(dispA[:, 2])
    tb = np.ascontiguousarray(dispB[:, 2])
    N_e = np.ascontiguousarray(f_local[:, 3])
    V_e = np.ascontiguousarray(f_local[:, 4])
    M1_e = np.ascontiguousarray(f_local[:, 2])
    M2_e = np.ascontiguousarray(f_local[:, 5])

    # host reduction: nodal_forces[n] = sum of f_global endpoint contributions
    nodal = np.zeros((N_NODES, 3), dtype=np.float32)
    for col in range(3):
        acc = np.bincount(nA, weights=f_global[:, col].astype(np.float64), minlength=N_NODES)
        acc += np.bincount(nB, weights=f_global[:, col + 3].astype(np.float64), minlength=N_NODES)
        nodal[:, col] = acc.astype(np.float32)

    return (nodal, f_global, f_local, d_local, d_global,
            N_e, V_e, M1_e, M2_e, u_l, ta, tb, phi, l0, c, s)
